# revision 15
# baseline (speedup 1.0000x reference)
"""Trainium2 Bass kernel for EvoAttn (B=2, L=2048, E=1024, H=16, D=64, causal,
multiplicative attention mask on q/k/v, fp32 in/out).

Sharding: batch*heads across 8 cores. Core c handles batch c//4, heads
[4*(c%4), 4*(c%4)+4). Each core computes its 4 heads' q/k/v projections
(column-parallel), full local attention, and a partial out-projection
(row-parallel). Partials (bf16) are summed on the host (unshard), bias added.

Layout notes (per core):
  xT   [128, 8, 2048]  host-packed (x[b]*mask).T chunked by 128-row groups
  wqT/wkT/wvT [128, 8, 256], woM [128, 2, 1024]  host-packed likewise
  qT/kT [256, 2048] in two partition blocks; head h at partitions 64*(h%2)..+63
  of block h//2. Scores are computed transposed (keys on partitions, queries on
  the free dim) so softmax needs no transposes: V is augmented with a ones
  column so the attn@V matmul also emits the softmax denominator. The V
  augmentation layout depends on head parity so each head's output lands on
  the partition half its yT slot needs:
    even head: [v(64) | ones(1) | zeros(63)] -> y rows 0..63,  denom row 64
    odd head:  [ones(1) | zeros(63) | v(64)] -> y rows 64..127, denom row 0

Scheduling (all aimed at keeping the PE stream dense - any stall re-cools the
PE p-state clock and slows every subsequent matmul):
  - scores/exp/attn@V run per 128-key chunk with a 2-chunk software pipeline:
    the PE emits scores(j+1), scores(j+2) before attnV(j), so the scalar
    engine's exp latency is hidden.
  - causal masking of partial (diagonal) chunks = exp then a 0/1 staircase
    multiply on the GpSimd engine (vector engine is loaded with psum casts).
  - softmax normalize chain (denominator row -> full-128 selector-matmul
    broadcast -> reciprocal -> multiply into yT) for head h is deferred one
    head; the out-projection of a finished query tile runs one 128-token
    block per head iteration of the next tile.
  - next tile's q/k/v projections are interleaved between attention heads.
"""

import os
from collections import deque

import numpy as np

B, L, E, H, D = 2, 2048, 1024, 16, 64
DLOC = E // 4          # local out dims per core (4 heads * 64)

_CACHE = {}
LAST_RESULTS = None


def _build_program():
    from contextlib import ExitStack

    import concourse.bacc as bacc
    import concourse.mybir as mybir
    import concourse.tile as tile

    f32 = mybir.dt.float32
    f32r = mybir.dt.float32r
    bf16 = mybir.dt.bfloat16
    Exp = mybir.ActivationFunctionType.Exp

    nc = bacc.Bacc("TRN2", target_bir_lowering=False, debug=False, num_devices=8)

    def dram_in(name, shape, dt):
        dd = f32 if dt in (f32, f32r) else dt
        ap = nc.dram_tensor(name, shape, dd, kind="ExternalInput").ap()
        return ap.bitcast(dt) if dt == f32r else ap

    xT = dram_in("xT", [128, 8, L], bf16)
    wqT = dram_in("wqT", [128, 8, DLOC], bf16)
    wkT = dram_in("wkT", [128, 8, DLOC], bf16)
    wvT = dram_in("wvT", [128, 8, DLOC], bf16)
    woM = dram_in("woM", [128, 2, E], bf16)
    # 0/1 causal staircase masks for the 4 partial chunk offsets
    pairmask = dram_in("pairmask", [128, 2048], bf16)
    vcones = dram_in("vcones", [128, 16], bf16)
    # bc selector: cols 0:128 broadcast row 64 (even heads), 128:256 row 0
    bcsel = dram_in("bcsel", [128, 256], f32r)
    outp = nc.dram_tensor("outp", [L, E], bf16, kind="ExternalOutput").ap()

    with (
        tile.TileContext(nc) as tc,
        ExitStack() as ctx,
        nc.allow_low_precision(reason="bf16 matmul inputs / bf16 partial out"),
    ):
        const_pool = ctx.enter_context(tc.tile_pool(name="const", bufs=1))
        w_pool = ctx.enter_context(tc.tile_pool(name="wp", bufs=1))
        qk_pool = ctx.enter_context(tc.tile_pool(name="qk", bufs=1))
        v_pool = ctx.enter_context(tc.tile_pool(name="vp", bufs=1))
        y_pool = ctx.enter_context(tc.tile_pool(name="yp", bufs=1))
        x_pool = ctx.enter_context(tc.tile_pool(name="xp", bufs=1))
        e_pool = ctx.enter_context(tc.tile_pool(name="ep", bufs=6))
        rb_pool = ctx.enter_context(tc.tile_pool(name="rb", bufs=3))
        ob_pool = ctx.enter_context(tc.tile_pool(name="ob", bufs=3))
        pp_psum = ctx.enter_context(tc.tile_pool(name="pp", bufs=2, space="PSUM"))
        sc_psum = ctx.enter_context(tc.tile_pool(name="sc", bufs=4, space="PSUM"))
        ya_psum = ctx.enter_context(tc.tile_pool(name="ya", bufs=2, space="PSUM"))

        # ---- persistent tiles ----
        pm_sb = const_pool.tile([128, 2048], bf16, tag="pm")
        bcsel_sb = const_pool.tile([128, 256], f32r, tag="bcsel")
        # declared f32 (f32r memset fails the ISA check); bitcast at use
        rrE = const_pool.tile([128, 512], f32, tag="rrE")
        rrO = const_pool.tile([128, 512], f32, tag="rrO")
        x_sb = x_pool.tile([128, 8, L], bf16, tag="x")
        q_sb = qk_pool.tile([128, 2, L], bf16, tag="q")
        k_sb = qk_pool.tile([128, 2, L], bf16, tag="k")
        v_sb = v_pool.tile([128, 16, 512], bf16, tag="v")
        v5 = v_sb.rearrange("p t (hp par c) -> p t hp par c", hp=2, par=2)
        yT_sb = y_pool.tile([128, 2, L], bf16, tag="y")
        wq_sb = w_pool.tile([128, 8, DLOC], bf16, tag="wq")
        wk_sb = w_pool.tile([128, 8, DLOC], bf16, tag="wk")
        wv_sb = w_pool.tile([128, 8, DLOC], bf16, tag="wv")
        wo_sb = w_pool.tile([128, 2, E], bf16, tag="wo")

        # ---- startup loads ----
        # reciprocal-row tiles zeroed once (only one row is ever rewritten;
        # the selector matmul multiplies the rest by 0, which must not be NaN)
        nc.vector.memset(rrE, 0.0)
        nc.vector.memset(rrO, 0.0)
        # v ones/zeros scaffold built by memsets (vector engine is idle at
        # startup and this avoids 2MB of DRAM reads): zeros everywhere, then
        # the per-head ones column (the v-value regions are overwritten by
        # the projection copies)
        nc.vector.memset(v_sb, 0.0)
        for h in range(4):
            col = 128 * h + (64 if h % 2 == 0 else 0)
            nc.scalar.dma_start(v_sb[:, :, col : col + 1], vcones)

        # x on the gpsimd queue (engine-blocking there, so tile 1 goes on
        # sync instead: gpsimd must run tile 0's mask-muls immediately)
        def emit_xdma(i, eng):
            tsl = slice(512 * i, 512 * i + 512)
            for c in range(8):
                eng.dma_start(x_sb[:, c, tsl], xT[:, c, tsl])

        emit_xdma(0, nc.gpsimd)
        # weights: q/k path on sync, v path + masks on scalar
        for c in range(8):
            nc.sync.dma_start(wq_sb[:, c, :], wqT[:, c, :])
        for c in range(8):
            nc.sync.dma_start(wk_sb[:, c, :], wkT[:, c, :])
        for c in range(8):
            nc.scalar.dma_start(wv_sb[:, c, :], wvT[:, c, :])
        for s in range(4):
            nc.scalar.dma_start(pm_sb[:, 512 * s : 512 * s + 512],
                                pairmask[:, 512 * s : 512 * s + 512])
        nc.scalar.dma_start(bcsel_sb, bcsel)
        emit_xdma(1, nc.sync)
        for c in range(2):
            nc.sync.dma_start(wo_sb[:, c, :], woM[:, c, :])

        # ---- helper closures ----
        def norm_closure(h, i, ya):
            blk, par = h // 2, h % 2
            p0 = 64 * par

            def norm():
                rr, dr = (rrE, 64) if par == 0 else (rrO, 0)
                sel = bcsel_sb[:, 128 * par : 128 * par + 128]
                # stage the raw denominator row (selector matmul rhs must be
                # SBUF), broadcast it to all 128 partitions with the selector
                # matmul, then reciprocal on full width
                nc.vector.tensor_copy(
                    rr[dr : dr + 1, :].bitcast(f32r), ya[dr : dr + 1, :])
                bc = pp_psum.tile([128, 512], f32, tag="pp")
                nc.tensor.matmul(bc, sel, rr[:, :].bitcast(f32r),
                                 start=True, stop=True)
                rbs = rb_pool.tile([128, 512], f32, tag="rbs")
                nc.vector.tensor_copy(rbs, bc)
                rb = rb_pool.tile([128, 512], f32, tag="rb")
                # (custom DVE op quirks on HW: PSUM reads and nonzero base
                # partition both produce garbage - keep it SBUF at offset 0)
                nc.vector.reciprocal_approx_fast(out=rb, in_=rbs)
                nc.vector.tensor_mul(
                    yT_sb[p0 : p0 + 64, blk, 512 * i : 512 * i + 512],
                    ya[p0 : p0 + 64, :],
                    rb[p0 : p0 + 64, :],
                )

            return norm

        def outproj_tb_closure(i, tb):
            tbg = 4 * i + tb

            def op():
                ob = ob_pool.tile([128, 1024], bf16, tag="o")
                for et in range(2):
                    ps = pp_psum.tile([128, 512], f32, tag="pp")
                    for c in range(2):
                        nc.tensor.matmul(
                            ps,
                            yT_sb[:, c, 128 * tbg : 128 * tbg + 128],
                            wo_sb[:, c, 512 * et : 512 * et + 512],
                            start=(c == 0),
                            stop=(c == 1),
                        )
                    dst = ob[:, 512 * et : 512 * et + 512]
                    if et == 0:
                        nc.vector.tensor_copy(dst, ps)
                        nc.scalar.dma_start(
                            outp[128 * tbg : 128 * tbg + 128, 0:512], dst)
                    else:
                        nc.scalar.copy(dst, ps)
                        nc.sync.dma_start(
                            outp[128 * tbg : 128 * tbg + 128, 512:1024], dst)

            return op

        def emit_proj_part(i, part):
            tsl = slice(512 * i, 512 * i + 512)
            if part in (0, 1):
                # qT/kT [256, 2048] = W_loc @ x.T
                w_sb, dst = ((wq_sb, q_sb), (wk_sb, k_sb))[part]
                for blk in range(2):
                    ps = pp_psum.tile([128, 512], f32, tag="pp")
                    for c in range(8):
                        nc.tensor.matmul(
                            ps,
                            w_sb[:, c, 128 * blk : 128 * blk + 128],
                            x_sb[:, c, tsl],
                            start=(c == 0),
                            stop=(c == 7),
                        )
                    nc.vector.tensor_copy(dst[:, blk, tsl], ps)
            else:
                # v natural [t, dims]: lhsT = xT chunk (stationary)
                for tb in range(4 * i + 2 * (part - 2), 4 * i + 2 * (part - 2) + 2):
                    ps = pp_psum.tile([128, 256], f32, tag="pp")
                    for c in range(8):
                        nc.tensor.matmul(
                            ps,
                            x_sb[:, c, 128 * tb : 128 * tb + 128],
                            wv_sb[:, c, :],
                            start=(c == 0),
                            stop=(c == 7),
                        )
                    psr = ps.rearrange("p (hp par c) -> p hp par c", hp=2, par=2)
                    nc.vector.tensor_copy(v5[:, tb, :, 0, 0:D], psr[:, :, 0, :])
                    nc.vector.tensor_copy(v5[:, tb, :, 1, D:128], psr[:, :, 1, :])

        def attention(i, h):
            """scores -> exp(+mask) -> attnV per 128-key chunk, with the PE
            running 2 chunks of scores ahead of attn@V."""
            blk, par = h // 2, h % 2
            p0 = 64 * par
            nch = 4 * i + 4
            ya = ya_psum.tile([128, 512], f32, tag="ya")

            def emit_attnv(j, o, e):
                nc.tensor.matmul(
                    ya[:, o:512],
                    v_sb[:, j, 128 * h : 128 * h + 128],
                    e[:, o:512],
                    start=(j == 0),
                    stop=(j == nch - 1),
                    skip_group_check=True,
                )

            pend = deque()
            for j in range(nch):
                s = j - 4 * i
                o = 128 * s if s > 0 else 0
                ps = sc_psum.tile([128, 512], f32, tag="sc")
                nc.tensor.matmul(
                    ps[:, o:512],
                    k_sb[p0 : p0 + 64, blk, 128 * j : 128 * j + 128],
                    q_sb[p0 : p0 + 64, blk, 512 * i + o : 512 * i + 512],
                    start=True,
                    stop=True,
                )
                e = e_pool.tile([128, 512], bf16, tag="e")
                nc.scalar.activation(e[:, o:512], ps[:, o:512], Exp, scale=0.125)
                if s >= 0:
                    # causal mask: zero the exp of future keys
                    nc.gpsimd.tensor_mul(
                        e[:, o:512], e[:, o:512],
                        pm_sb[:, 512 * s + o : 512 * s + 512],
                    )
                pend.append((j, o, e))
                if len(pend) > 2:
                    emit_attnv(*pend.popleft())
            while pend:
                emit_attnv(*pend.popleft())
            return ya

        # ---- main loop ----
        for part in range(4):
            emit_proj_part(0, part)
        pending_norm = None
        pending_out = deque()
        for i in range(4):
            if 0 < i < 3:
                emit_xdma(i + 1, nc.gpsimd)
            for h in range(4):
                ya = attention(i, h)
                # deferred work: never let the normalize chain or out-proj
                # head-of-line-block the PE stream of the current head
                if pending_norm is not None:
                    pending_norm()
                pending_norm = norm_closure(h, i, ya)
                if i < 3:
                    emit_proj_part(i + 1, h)
                # pop only out-proj blocks queued in EARLIER iterations: a
                # tile's tb0 must wait for its own head 3's normalize
                if pending_out:
                    pending_out.popleft()()
                if h == 3:
                    for tb in range(4):
                        pending_out.append(outproj_tb_closure(i, tb))
        pending_norm()
        while pending_out:
            pending_out.popleft()()

    nc.compile()
    return nc


def _get_program():
    if "nc" not in _CACHE:
        _CACHE["nc"] = _build_program()
    return _CACHE["nc"]


def _host_consts():
    import ml_dtypes

    bf16 = ml_dtypes.bfloat16
    # pairmask[s]: mask(o=128s)[p, t] = 1 iff query t >= key p + 128s
    p = np.arange(128)[:, None]
    t = np.arange(512)[None, :]
    pairmask = np.zeros((128, 2048), dtype=np.float32)
    for s in range(4):
        pairmask[:, 512 * s : 512 * s + 512] = (t >= p + 128 * s)
    bcsel = np.zeros((128, 256), dtype=np.float32)
    bcsel[64, 0:128] = 1.0   # even heads: broadcast denom row 64
    bcsel[0, 128:256] = 1.0  # odd heads: broadcast denom row 0
    return {
        "pairmask": pairmask.astype(bf16),
        "vcones": np.ones((128, 16), dtype=np.float32).astype(bf16),
        "bcsel": bcsel,
    }


def _enable_trace_support():
    """Best-effort: make trace=True work in this container (NTFF hook shim +
    disable artifact upload). No-op if anything is missing."""
    import sys
    import types

    try:
        import concourse.bass_utils as bu

        bu.upload_artifacts = lambda tmpdir: tmpdir
        try:
            from antenv.axon_hooks import get_axon_ntff_profile_hook  # noqa: F401

            return True
        except ImportError:
            pass
        import antenv
        from trn_agent_boot.trn_boot import _ntff_profile_via_ctypes

        hook = _ntff_profile_via_ctypes("/opt/axon/libaxon_pjrt.so")
        mod = types.ModuleType("antenv.axon_hooks")
        state = {"hook": hook}
        mod.get_axon_ntff_profile_hook = lambda: state["hook"]
        mod.set_axon_ntff_profile_hook = lambda h: state.__setitem__("hook", h)
        sys.modules["antenv.axon_hooks"] = mod
        antenv.axon_hooks = mod
        return hook is not None
    except Exception:
        return False


def kernel(x, attention_mask, Wq, Wk, Wv, Wo, bo):
    global LAST_RESULTS
    import ml_dtypes
    from concourse.bass_utils import run_bass_kernel_spmd

    x = np.asarray(x, dtype=np.float32)
    attention_mask = np.asarray(attention_mask, dtype=np.float32)
    Wq = np.asarray(Wq, dtype=np.float32)
    Wk = np.asarray(Wk, dtype=np.float32)
    Wv = np.asarray(Wv, dtype=np.float32)
    Wo = np.asarray(Wo, dtype=np.float32)
    bo = np.asarray(bo, dtype=np.float32)

    nc = _get_program()
    bf16 = ml_dtypes.bfloat16

    def pack(a, chunks):
        # [128*chunks, F] -> [128, chunks, F] grouped by 128-row chunk
        f = a.shape[1]
        return np.ascontiguousarray(
            a.reshape(chunks, 128, f).transpose(1, 0, 2)).astype(bf16)

    # host-side shard prep
    xm = x * attention_mask[:, :, None]
    xTs = [pack(xm[b].T, 8) for b in range(B)]
    consts = _host_consts()

    in_maps = []
    for core in range(8):
        b, g = divmod(core, 4)
        sl = slice(DLOC * g, DLOC * g + DLOC)
        in_maps.append(
            {
                "xT": xTs[b],
                "wqT": pack(np.ascontiguousarray(Wq[sl]).T, 8),
                "wkT": pack(np.ascontiguousarray(Wk[sl]).T, 8),
                "wvT": pack(np.ascontiguousarray(Wv[sl]).T, 8),
                "woM": pack(np.ascontiguousarray(Wo[:, sl]).T, 2),
                **consts,
            }
        )

    trace = bool(int(os.environ.get("KERNEL_TRACE", "0")))
    if trace:
        trace = _enable_trace_support()
    res = run_bass_kernel_spmd(nc, in_maps, core_ids=list(range(8)), trace=trace)
    LAST_RESULTS = res

    out = np.zeros((B, L, E), dtype=np.float32)
    for core in range(8):
        out[core // 4] += res.results[core]["outp"].astype(np.float32)
    out += bo
    return out


# revision 16
# speedup vs baseline: 1.0370x; 1.0370x over previous
"""Trainium2 Bass kernel for EvoAttn (B=2, L=2048, E=1024, H=16, D=64, causal,
multiplicative attention mask on q/k/v, fp32 in/out).

Sharding: batch*heads across 8 cores. Core c handles batch c//4, heads
[4*(c%4), 4*(c%4)+4). Each core computes its 4 heads' q/k/v projections
(column-parallel), full local attention, and a partial out-projection
(row-parallel). Partials (bf16) are summed on the host (unshard), bias added.

Layout notes (per core):
  xT   [128, 8, 2048]  host-packed (x[b]*mask).T chunked by 128-row groups
  wqT/wkT/wvT [128, 8, 256], woM [128, 2, 1024]  host-packed likewise
  qT/kT [256, 2048] in two partition blocks; head h at partitions 64*(h%2)..+63
  of block h//2. Scores are computed transposed (keys on partitions, queries on
  the free dim) so softmax needs no transposes: V is augmented with a ones
  column so the attn@V matmul also emits the softmax denominator. The V
  augmentation layout depends on head parity so each head's output lands on
  the partition half its yT slot needs:
    even head: [v(64) | ones(1) | zeros(63)] -> y rows 0..63,  denom row 64
    odd head:  [ones(1) | zeros(63) | v(64)] -> y rows 64..127, denom row 0

Scheduling (all aimed at keeping the PE stream dense - any stall re-cools the
PE p-state clock and slows every subsequent matmul):
  - scores/exp/attn@V run per 128-key chunk with a 2-chunk software pipeline:
    the PE emits scores(j+1), scores(j+2) before attnV(j), so the scalar
    engine's exp latency is hidden.
  - causal masking of partial (diagonal) chunks = exp then a 0/1 staircase
    multiply on the GpSimd engine (vector engine is loaded with psum casts).
  - softmax normalize chain (denominator row -> full-128 selector-matmul
    broadcast -> reciprocal -> multiply into yT) for head h is deferred one
    head; the out-projection of a finished query tile runs one 128-token
    block per head iteration of the next tile.
  - next tile's q/k/v projections are interleaved between attention heads.
"""

import os
from collections import deque

import numpy as np

B, L, E, H, D = 2, 2048, 1024, 16, 64
DLOC = E // 4          # local out dims per core (4 heads * 64)

_CACHE = {}
LAST_RESULTS = None


def _build_program():
    from contextlib import ExitStack

    import concourse.bacc as bacc
    import concourse.mybir as mybir
    import concourse.tile as tile

    f32 = mybir.dt.float32
    f32r = mybir.dt.float32r
    bf16 = mybir.dt.bfloat16
    Exp = mybir.ActivationFunctionType.Exp

    nc = bacc.Bacc("TRN2", target_bir_lowering=False, debug=False, num_devices=8)

    def dram_in(name, shape, dt):
        dd = f32 if dt in (f32, f32r) else dt
        ap = nc.dram_tensor(name, shape, dd, kind="ExternalInput").ap()
        return ap.bitcast(dt) if dt == f32r else ap

    xT = dram_in("xT", [128, 8, L], bf16)
    wqT = dram_in("wqT", [128, 8, DLOC], bf16)
    wkT = dram_in("wkT", [128, 8, DLOC], bf16)
    wvT = dram_in("wvT", [128, 8, DLOC], bf16)
    woM = dram_in("woM", [128, 2, E], bf16)
    # 0/1 causal staircase masks for the 4 partial chunk offsets
    pairmask = dram_in("pairmask", [128, 2048], bf16)
    vcones = dram_in("vcones", [128, 16], bf16)
    # bc selector: cols 0:128 broadcast row 64 (even heads), 128:256 row 0
    bcsel = dram_in("bcsel", [128, 256], f32r)
    outp = nc.dram_tensor("outp", [L, E], bf16, kind="ExternalOutput").ap()

    with (
        tile.TileContext(nc) as tc,
        ExitStack() as ctx,
        nc.allow_low_precision(reason="bf16 matmul inputs / bf16 partial out"),
    ):
        const_pool = ctx.enter_context(tc.tile_pool(name="const", bufs=1))
        w_pool = ctx.enter_context(tc.tile_pool(name="wp", bufs=1))
        qk_pool = ctx.enter_context(tc.tile_pool(name="qk", bufs=1))
        v_pool = ctx.enter_context(tc.tile_pool(name="vp", bufs=1))
        y_pool = ctx.enter_context(tc.tile_pool(name="yp", bufs=1))
        x_pool = ctx.enter_context(tc.tile_pool(name="xp", bufs=1))
        e_pool = ctx.enter_context(tc.tile_pool(name="ep", bufs=6))
        rb_pool = ctx.enter_context(tc.tile_pool(name="rb", bufs=3))
        ob_pool = ctx.enter_context(tc.tile_pool(name="ob", bufs=3))
        pp_psum = ctx.enter_context(tc.tile_pool(name="pp", bufs=2, space="PSUM"))
        sc_psum = ctx.enter_context(tc.tile_pool(name="sc", bufs=4, space="PSUM"))
        ya_psum = ctx.enter_context(tc.tile_pool(name="ya", bufs=2, space="PSUM"))

        # ---- persistent tiles ----
        pm_sb = const_pool.tile([128, 2048], bf16, tag="pm")
        bcsel_sb = const_pool.tile([128, 256], f32r, tag="bcsel")
        # declared f32 (f32r memset fails the ISA check); bitcast at use
        rrE = const_pool.tile([128, 512], f32, tag="rrE")
        rrO = const_pool.tile([128, 512], f32, tag="rrO")
        x_sb = x_pool.tile([128, 8, L], bf16, tag="x")
        q_sb = qk_pool.tile([128, 2, L], bf16, tag="q")
        k_sb = qk_pool.tile([128, 2, L], bf16, tag="k")
        v_sb = v_pool.tile([128, 16, 512], bf16, tag="v")
        v5 = v_sb.rearrange("p t (hp par c) -> p t hp par c", hp=2, par=2)
        yT_sb = y_pool.tile([128, 2, L], bf16, tag="y")
        wq_sb = w_pool.tile([128, 8, DLOC], bf16, tag="wq")
        wk_sb = w_pool.tile([128, 8, DLOC], bf16, tag="wk")
        wv_sb = w_pool.tile([128, 8, DLOC], bf16, tag="wv")
        wo_sb = w_pool.tile([128, 2, E], bf16, tag="wo")

        # ---- startup loads ----
        # reciprocal-row tiles zeroed once (only one row is ever rewritten;
        # the selector matmul multiplies the rest by 0, which must not be NaN)
        nc.vector.memset(rrE, 0.0)
        nc.vector.memset(rrO, 0.0)
        # v ones/zeros scaffold built by memsets (vector engine is idle at
        # startup and this avoids 2MB of DRAM reads): zeros everywhere, then
        # the per-head ones column (the v-value regions are overwritten by
        # the projection copies)
        nc.vector.memset(v_sb, 0.0)
        for h in range(4):
            col = 128 * h + (64 if h % 2 == 0 else 0)
            nc.scalar.dma_start(v_sb[:, :, col : col + 1], vcones)

        # x on the gpsimd queue (engine-blocking there, so tile 1 goes on
        # sync instead: gpsimd must run tile 0's mask-muls immediately)
        def emit_xdma(i, eng):
            tsl = slice(512 * i, 512 * i + 512)
            for c in range(8):
                eng.dma_start(x_sb[:, c, tsl], xT[:, c, tsl])

        emit_xdma(0, nc.gpsimd)
        # weights: q/k path on sync, v path + masks on scalar
        for c in range(8):
            nc.sync.dma_start(wq_sb[:, c, :], wqT[:, c, :])
        for c in range(8):
            nc.sync.dma_start(wk_sb[:, c, :], wkT[:, c, :])
        for c in range(8):
            nc.scalar.dma_start(wv_sb[:, c, :], wvT[:, c, :])
        for s in range(4):
            nc.scalar.dma_start(pm_sb[:, 512 * s : 512 * s + 512],
                                pairmask[:, 512 * s : 512 * s + 512])
        nc.scalar.dma_start(bcsel_sb, bcsel)
        emit_xdma(1, nc.sync)
        for c in range(2):
            nc.sync.dma_start(wo_sb[:, c, :], woM[:, c, :])

        # ---- helper closures ----
        def norm_closure(h, i, ya):
            blk, par = h // 2, h % 2
            p0 = 64 * par

            def norm():
                rr, dr = (rrE, 64) if par == 0 else (rrO, 0)
                sel = bcsel_sb[:, 128 * par : 128 * par + 128]
                # stage the raw denominator row (selector matmul rhs must be
                # SBUF), broadcast it to all 128 partitions with the selector
                # matmul, then reciprocal on full width
                nc.vector.tensor_copy(
                    rr[dr : dr + 1, :].bitcast(f32r), ya[dr : dr + 1, :])
                bc = pp_psum.tile([128, 512], f32, tag="pp")
                nc.tensor.matmul(bc, sel, rr[:, :].bitcast(f32r),
                                 start=True, stop=True)
                rbs = rb_pool.tile([128, 512], f32, tag="rbs")
                nc.vector.tensor_copy(rbs, bc)
                rb = rb_pool.tile([128, 512], f32, tag="rb")
                # (custom DVE op quirks on HW: PSUM reads and nonzero base
                # partition both produce garbage - keep it SBUF at offset 0)
                nc.vector.reciprocal_approx_fast(out=rb, in_=rbs)
                nc.vector.tensor_mul(
                    yT_sb[p0 : p0 + 64, blk, 512 * i : 512 * i + 512],
                    ya[p0 : p0 + 64, :],
                    rb[p0 : p0 + 64, :],
                )

            return norm

        def outproj_tb_closure(i, tb):
            tbg = 4 * i + tb

            def op():
                ob = ob_pool.tile([128, 1024], bf16, tag="o")
                for et in range(2):
                    ps = pp_psum.tile([128, 512], f32, tag="pp")
                    for c in range(2):
                        nc.tensor.matmul(
                            ps,
                            yT_sb[:, c, 128 * tbg : 128 * tbg + 128],
                            wo_sb[:, c, 512 * et : 512 * et + 512],
                            start=(c == 0),
                            stop=(c == 1),
                        )
                    dst = ob[:, 512 * et : 512 * et + 512]
                    if et == 0:
                        nc.vector.tensor_copy(dst, ps)
                        nc.scalar.dma_start(
                            outp[128 * tbg : 128 * tbg + 128, 0:512], dst)
                    else:
                        nc.scalar.copy(dst, ps)
                        nc.sync.dma_start(
                            outp[128 * tbg : 128 * tbg + 128, 512:1024], dst)

            return op

        def emit_proj_part(i, part):
            tsl = slice(512 * i, 512 * i + 512)
            if part in (0, 1):
                # qT/kT [256, 2048] = W_loc @ x.T
                w_sb, dst = ((wq_sb, q_sb), (wk_sb, k_sb))[part]
                for blk in range(2):
                    ps = pp_psum.tile([128, 512], f32, tag="pp")
                    for c in range(8):
                        nc.tensor.matmul(
                            ps,
                            w_sb[:, c, 128 * blk : 128 * blk + 128],
                            x_sb[:, c, tsl],
                            start=(c == 0),
                            stop=(c == 7),
                        )
                    nc.vector.tensor_copy(dst[:, blk, tsl], ps)
            else:
                # v natural [t, dims]: lhsT = xT chunk (stationary)
                for tb in range(4 * i + 2 * (part - 2), 4 * i + 2 * (part - 2) + 2):
                    ps = pp_psum.tile([128, 256], f32, tag="pp")
                    for c in range(8):
                        nc.tensor.matmul(
                            ps,
                            x_sb[:, c, 128 * tb : 128 * tb + 128],
                            wv_sb[:, c, :],
                            start=(c == 0),
                            stop=(c == 7),
                        )
                    psr = ps.rearrange("p (hp par c) -> p hp par c", hp=2, par=2)
                    nc.vector.tensor_copy(v5[:, tb, :, 0, 0:D], psr[:, :, 0, :])
                    nc.vector.tensor_copy(v5[:, tb, :, 1, D:128], psr[:, :, 1, :])

        def attention(i, h):
            """scores -> exp(+mask) -> attnV per 128-key chunk, with the PE
            running 2 chunks of scores ahead of attn@V."""
            blk, par = h // 2, h % 2
            p0 = 64 * par
            nch = 4 * i + 4
            ya = ya_psum.tile([128, 512], f32, tag="ya")

            def emit_attnv(j, o, e):
                nc.tensor.matmul(
                    ya[:, o:512],
                    v_sb[:, j, 128 * h : 128 * h + 128],
                    e[:, o:512],
                    start=(j == 0),
                    stop=(j == nch - 1),
                    skip_group_check=True,
                )

            pend = deque()
            for j in range(nch):
                s = j - 4 * i
                o = 128 * s if s > 0 else 0
                ps = sc_psum.tile([128, 512], f32, tag="sc")
                nc.tensor.matmul(
                    ps[:, o:512],
                    k_sb[p0 : p0 + 64, blk, 128 * j : 128 * j + 128],
                    q_sb[p0 : p0 + 64, blk, 512 * i + o : 512 * i + 512],
                    start=True,
                    stop=True,
                )
                e = e_pool.tile([128, 512], bf16, tag="e")
                nc.scalar.activation(e[:, o:512], ps[:, o:512], Exp, scale=0.125)
                if s >= 0:
                    # causal mask: zero the exp of future keys
                    nc.vector.tensor_mul(
                        e[:, o:512], e[:, o:512],
                        pm_sb[:, 512 * s + o : 512 * s + 512],
                    )
                pend.append((j, o, e))
                if len(pend) > 3:
                    emit_attnv(*pend.popleft())
            while pend:
                emit_attnv(*pend.popleft())
            return ya

        # ---- main loop ----
        for part in range(4):
            emit_proj_part(0, part)
        pending_norm = None
        pending_out = deque()
        for i in range(4):
            if 0 < i < 3:
                emit_xdma(i + 1, nc.gpsimd)
            for h in range(4):
                ya = attention(i, h)
                # deferred work: never let the normalize chain or out-proj
                # head-of-line-block the PE stream of the current head
                if pending_norm is not None:
                    pending_norm()
                pending_norm = norm_closure(h, i, ya)
                if i < 3:
                    emit_proj_part(i + 1, h)
                # pop only out-proj blocks queued in EARLIER iterations: a
                # tile's tb0 must wait for its own head 3's normalize
                if pending_out:
                    pending_out.popleft()()
                if h == 3:
                    for tb in range(4):
                        pending_out.append(outproj_tb_closure(i, tb))
        pending_norm()
        while pending_out:
            pending_out.popleft()()

    nc.compile()
    return nc


def _get_program():
    if "nc" not in _CACHE:
        _CACHE["nc"] = _build_program()
    return _CACHE["nc"]


def _host_consts():
    import ml_dtypes

    bf16 = ml_dtypes.bfloat16
    # pairmask[s]: mask(o=128s)[p, t] = 1 iff query t >= key p + 128s
    p = np.arange(128)[:, None]
    t = np.arange(512)[None, :]
    pairmask = np.zeros((128, 2048), dtype=np.float32)
    for s in range(4):
        pairmask[:, 512 * s : 512 * s + 512] = (t >= p + 128 * s)
    bcsel = np.zeros((128, 256), dtype=np.float32)
    bcsel[64, 0:128] = 1.0   # even heads: broadcast denom row 64
    bcsel[0, 128:256] = 1.0  # odd heads: broadcast denom row 0
    return {
        "pairmask": pairmask.astype(bf16),
        "vcones": np.ones((128, 16), dtype=np.float32).astype(bf16),
        "bcsel": bcsel,
    }


def _enable_trace_support():
    """Best-effort: make trace=True work in this container (NTFF hook shim +
    disable artifact upload). No-op if anything is missing."""
    import sys
    import types

    try:
        import concourse.bass_utils as bu

        bu.upload_artifacts = lambda tmpdir: tmpdir
        try:
            from antenv.axon_hooks import get_axon_ntff_profile_hook  # noqa: F401

            return True
        except ImportError:
            pass
        import antenv
        from trn_agent_boot.trn_boot import _ntff_profile_via_ctypes

        hook = _ntff_profile_via_ctypes("/opt/axon/libaxon_pjrt.so")
        mod = types.ModuleType("antenv.axon_hooks")
        state = {"hook": hook}
        mod.get_axon_ntff_profile_hook = lambda: state["hook"]
        mod.set_axon_ntff_profile_hook = lambda h: state.__setitem__("hook", h)
        sys.modules["antenv.axon_hooks"] = mod
        antenv.axon_hooks = mod
        return hook is not None
    except Exception:
        return False


def kernel(x, attention_mask, Wq, Wk, Wv, Wo, bo):
    global LAST_RESULTS
    import ml_dtypes
    from concourse.bass_utils import run_bass_kernel_spmd

    x = np.asarray(x, dtype=np.float32)
    attention_mask = np.asarray(attention_mask, dtype=np.float32)
    Wq = np.asarray(Wq, dtype=np.float32)
    Wk = np.asarray(Wk, dtype=np.float32)
    Wv = np.asarray(Wv, dtype=np.float32)
    Wo = np.asarray(Wo, dtype=np.float32)
    bo = np.asarray(bo, dtype=np.float32)

    nc = _get_program()
    bf16 = ml_dtypes.bfloat16

    def pack(a, chunks):
        # [128*chunks, F] -> [128, chunks, F] grouped by 128-row chunk
        f = a.shape[1]
        return np.ascontiguousarray(
            a.reshape(chunks, 128, f).transpose(1, 0, 2)).astype(bf16)

    # host-side shard prep
    xm = x * attention_mask[:, :, None]
    xTs = [pack(xm[b].T, 8) for b in range(B)]
    consts = _host_consts()

    in_maps = []
    for core in range(8):
        b, g = divmod(core, 4)
        sl = slice(DLOC * g, DLOC * g + DLOC)
        in_maps.append(
            {
                "xT": xTs[b],
                "wqT": pack(np.ascontiguousarray(Wq[sl]).T, 8),
                "wkT": pack(np.ascontiguousarray(Wk[sl]).T, 8),
                "wvT": pack(np.ascontiguousarray(Wv[sl]).T, 8),
                "woM": pack(np.ascontiguousarray(Wo[:, sl]).T, 2),
                **consts,
            }
        )

    trace = bool(int(os.environ.get("KERNEL_TRACE", "0")))
    if trace:
        trace = _enable_trace_support()
    res = run_bass_kernel_spmd(nc, in_maps, core_ids=list(range(8)), trace=trace)
    LAST_RESULTS = res

    out = np.zeros((B, L, E), dtype=np.float32)
    for core in range(8):
        out[core // 4] += res.results[core]["outp"].astype(np.float32)
    out += bo
    return out


# revision 23
# speedup vs baseline: 1.1928x; 1.1503x over previous
"""Trainium2 Bass kernel for EvoAttn (B=2, L=2048, E=1024, H=16, D=64, causal,
multiplicative attention mask on q/k/v, fp32 in/out).

Sharding: batch*heads across 8 cores. Core c handles batch c//4, heads
[4*(c%4), 4*(c%4)+4). Each core computes its 4 heads' q/k/v projections
(column-parallel), full local attention, and a partial out-projection
(row-parallel). Partials (bf16) are summed on the host (unshard), bias added.

Layout notes (per core):
  xT   [128, 8, 2048]  host-packed (x[b]*mask).T chunked by 128-row groups
  wqT/wkT/wvT [128, 8, 256], woM [128, 2, 1024]  host-packed likewise
  qT/kT [256, 2048] in two partition blocks; head h at partitions 64*(h%2)..+63
  of block h//2. Scores are computed transposed (keys on partitions, queries on
  the free dim) so softmax needs no transposes: V is augmented with a ones
  column so the attn@V matmul also emits the softmax denominator. The V
  augmentation layout depends on head parity so each head's output lands on
  the partition half its yT slot needs:
    even head: [v(64) | ones(1) | zeros(63)] -> y rows 0..63,  denom row 64
    odd head:  [ones(1) | zeros(63) | v(64)] -> y rows 64..127, denom row 0

Scheduling (all aimed at keeping the PE stream dense - any stall re-cools the
PE p-state clock and slows every subsequent matmul):
  - scores/exp/attn@V run per 128-key chunk with a 2-chunk software pipeline:
    the PE emits scores(j+1), scores(j+2) before attnV(j), so the scalar
    engine's exp latency is hidden.
  - causal masking of partial (diagonal) chunks = exp then a 0/1 staircase
    multiply on the GpSimd engine (vector engine is loaded with psum casts).
  - softmax normalize chain (denominator row -> full-128 selector-matmul
    broadcast -> reciprocal -> multiply into yT) for head h is deferred one
    head; the out-projection of a finished query tile runs one 128-token
    block per head iteration of the next tile.
  - next tile's q/k/v projections are interleaved between attention heads.
"""

import os
from collections import deque

import numpy as np

B, L, E, H, D = 2, 2048, 1024, 16, 64
DLOC = E // 4          # local out dims per core (4 heads * 64)

_CACHE = {}
LAST_RESULTS = None


def _build_program():
    from contextlib import ExitStack

    import concourse.bacc as bacc
    import concourse.mybir as mybir
    import concourse.tile as tile

    f32 = mybir.dt.float32
    f32r = mybir.dt.float32r
    bf16 = mybir.dt.bfloat16
    Exp = mybir.ActivationFunctionType.Exp

    nc = bacc.Bacc("TRN2", target_bir_lowering=False, debug=False, num_devices=8)

    def dram_in(name, shape, dt):
        dd = f32 if dt in (f32, f32r) else dt
        ap = nc.dram_tensor(name, shape, dd, kind="ExternalInput").ap()
        return ap.bitcast(dt) if dt == f32r else ap

    xT = dram_in("xT", [128, 8, L], bf16)
    wqT = dram_in("wqT", [128, 8, DLOC], bf16)
    wkT = dram_in("wkT", [128, 8, DLOC], bf16)
    wvT = dram_in("wvT", [128, 8, DLOC], bf16)
    woM = dram_in("woM", [128, 2, E], bf16)
    # 0/1 causal staircase masks for the 4 partial chunk offsets
    pairmask = dram_in("pairmask", [128, 2048], bf16)
    vcones = dram_in("vcones", [128, 16], bf16)
    # bc selector: cols 0:128 broadcast row 64 (even heads), 128:256 row 0
    bcsel = dram_in("bcsel", [128, 256], f32r)
    outp = nc.dram_tensor("outp", [L, E], bf16, kind="ExternalOutput").ap()

    with (
        tile.TileContext(nc) as tc,
        ExitStack() as ctx,
        nc.allow_low_precision(reason="bf16 matmul inputs / bf16 partial out"),
    ):
        const_pool = ctx.enter_context(tc.tile_pool(name="const", bufs=1))
        w_pool = ctx.enter_context(tc.tile_pool(name="wp", bufs=1))
        qk_pool = ctx.enter_context(tc.tile_pool(name="qk", bufs=1))
        v_pool = ctx.enter_context(tc.tile_pool(name="vp", bufs=1))
        y_pool = ctx.enter_context(tc.tile_pool(name="yp", bufs=1))
        x_pool = ctx.enter_context(tc.tile_pool(name="xp", bufs=1))
        e_pool = ctx.enter_context(tc.tile_pool(name="ep", bufs=6))
        rb_pool = ctx.enter_context(tc.tile_pool(name="rb", bufs=3))
        ob_pool = ctx.enter_context(tc.tile_pool(name="ob", bufs=3))
        pp_psum = ctx.enter_context(tc.tile_pool(name="pp", bufs=2, space="PSUM"))
        sc_psum = ctx.enter_context(tc.tile_pool(name="sc", bufs=2, space="PSUM"))
        ya_psum = ctx.enter_context(tc.tile_pool(name="ya", bufs=2, space="PSUM"))

        # ---- persistent tiles ----
        pm_sb = const_pool.tile([128, 2048], bf16, tag="pm")
        bcsel_sb = const_pool.tile([128, 256], f32r, tag="bcsel")
        # declared f32 (f32r memset fails the ISA check); bitcast at use
        rrE = const_pool.tile([128, 512], f32, tag="rrE")
        rrO = const_pool.tile([128, 512], f32, tag="rrO")
        x_sb = x_pool.tile([128, 8, L], bf16, tag="x")
        q_sb = qk_pool.tile([128, 2, L], bf16, tag="q")
        k_sb = qk_pool.tile([128, 2, L], bf16, tag="k")
        v_sb = v_pool.tile([128, 16, 512], bf16, tag="v")
        v5 = v_sb.rearrange("p t (hp par c) -> p t hp par c", hp=2, par=2)
        yT_sb = y_pool.tile([128, 2, L], bf16, tag="y")
        wq_sb = w_pool.tile([128, 8, DLOC], bf16, tag="wq")
        wk_sb = w_pool.tile([128, 8, DLOC], bf16, tag="wk")
        wv_sb = w_pool.tile([128, 8, DLOC], bf16, tag="wv")
        wo_sb = w_pool.tile([128, 2, E], bf16, tag="wo")

        # ---- startup loads ----
        # reciprocal-row tiles zeroed once (only one row is ever rewritten;
        # the selector matmul multiplies the rest by 0, which must not be NaN)
        nc.vector.memset(rrE, 0.0)
        nc.vector.memset(rrO, 0.0)
        # v ones/zeros scaffold built by memsets (vector engine is idle at
        # startup and this avoids 2MB of DRAM reads): zeros everywhere, then
        # the per-head ones column (the v-value regions are overwritten by
        # the projection copies)
        nc.vector.memset(v_sb, 0.0)
        for h in range(4):
            col = 128 * h + (64 if h % 2 == 0 else 0)
            nc.scalar.dma_start(v_sb[:, :, col : col + 1], vcones)

        # x on the gpsimd queue (engine-blocking there, so tile 1 goes on
        # sync instead: gpsimd must run tile 0's mask-muls immediately)
        def emit_xdma(i, eng):
            tsl = slice(512 * i, 512 * i + 512)
            for c in range(8):
                eng.dma_start(x_sb[:, c, tsl], xT[:, c, tsl])

        emit_xdma(0, nc.gpsimd)
        # weights: q/k path on sync, v path + masks on scalar
        for c in range(8):
            nc.sync.dma_start(wq_sb[:, c, :], wqT[:, c, :])
        for c in range(8):
            nc.sync.dma_start(wk_sb[:, c, :], wkT[:, c, :])
        for c in range(8):
            nc.scalar.dma_start(wv_sb[:, c, :], wvT[:, c, :])
        for s in range(4):
            nc.scalar.dma_start(pm_sb[:, 512 * s : 512 * s + 512],
                                pairmask[:, 512 * s : 512 * s + 512])
        nc.scalar.dma_start(bcsel_sb, bcsel)

        # ---- helper closures ----
        def norm_closure(h, i, ya):
            blk, par = h // 2, h % 2
            p0 = 64 * par

            def norm():
                rr, dr = (rrE, 64) if par == 0 else (rrO, 0)
                sel = bcsel_sb[:, 128 * par : 128 * par + 128]
                # stage the raw denominator row (selector matmul rhs must be
                # SBUF), broadcast it to all 128 partitions with the selector
                # matmul, then reciprocal on full width
                nc.vector.tensor_copy(
                    rr[dr : dr + 1, :].bitcast(f32r), ya[dr : dr + 1, :])
                bc = pp_psum.tile([128, 512], f32, tag="pp")
                nc.tensor.matmul(bc, sel, rr[:, :].bitcast(f32r),
                                 start=True, stop=True)
                rbs = rb_pool.tile([128, 512], f32, tag="rbs")
                nc.vector.tensor_copy(rbs, bc)
                rb = rb_pool.tile([128, 512], f32, tag="rb")
                # (custom DVE op quirks on HW: PSUM reads and nonzero base
                # partition both produce garbage - keep it SBUF at offset 0)
                nc.vector.reciprocal_approx_fast(out=rb, in_=rbs)
                nc.vector.tensor_mul(
                    yT_sb[p0 : p0 + 64, blk, 512 * i : 512 * i + 512],
                    ya[p0 : p0 + 64, :],
                    rb[p0 : p0 + 64, :],
                )

            return norm

        def outproj_tb_closure(i, tb):
            tbg = 4 * i + tb

            def op():
                ob = ob_pool.tile([128, 1024], bf16, tag="o")
                for et in range(2):
                    ps = pp_psum.tile([128, 512], f32, tag="pp")
                    for c in range(2):
                        nc.tensor.matmul(
                            ps,
                            yT_sb[:, c, 128 * tbg : 128 * tbg + 128],
                            wo_sb[:, c, 512 * et : 512 * et + 512],
                            start=(c == 0),
                            stop=(c == 1),
                        )
                    dst = ob[:, 512 * et : 512 * et + 512]
                    if et == 0:
                        nc.vector.tensor_copy(dst, ps)
                        nc.scalar.dma_start(
                            outp[128 * tbg : 128 * tbg + 128, 0:512], dst)
                    else:
                        nc.scalar.copy(dst, ps)
                        nc.sync.dma_start(
                            outp[128 * tbg : 128 * tbg + 128, 512:1024], dst)

            return op

        def emit_proj_part(i, part):
            tsl = slice(512 * i, 512 * i + 512)
            if part in (0, 1):
                # qT/kT [256, 2048] = W_loc @ x.T
                w_sb, dst = ((wq_sb, q_sb), (wk_sb, k_sb))[part]
                for blk in range(2):
                    ps = pp_psum.tile([128, 512], f32, tag="pp")
                    for c in range(8):
                        nc.tensor.matmul(
                            ps,
                            w_sb[:, c, 128 * blk : 128 * blk + 128],
                            x_sb[:, c, tsl],
                            start=(c == 0),
                            stop=(c == 7),
                        )
                    nc.vector.tensor_copy(dst[:, blk, tsl], ps)
            else:
                # v natural [t, dims]: lhsT = xT chunk (stationary)
                for tb in range(4 * i + 2 * (part - 2), 4 * i + 2 * (part - 2) + 2):
                    ps = pp_psum.tile([128, 256], f32, tag="pp")
                    for c in range(8):
                        nc.tensor.matmul(
                            ps,
                            x_sb[:, c, 128 * tb : 128 * tb + 128],
                            wv_sb[:, c, :],
                            start=(c == 0),
                            stop=(c == 7),
                        )
                    psr = ps.rearrange("p (hp par c) -> p hp par c", hp=2, par=2)
                    nc.vector.tensor_copy(v5[:, tb, :, 0, 0:D], psr[:, :, 0, :])
                    nc.vector.tensor_copy(v5[:, tb, :, 1, D:128], psr[:, :, 1, :])

        def attention(i, h):
            """scores -> exp(+mask) -> attnV per 128-key chunk, with the PE
            running 2 chunks of scores ahead of attn@V."""
            blk, par = h // 2, h % 2
            p0 = 64 * par
            nch = 4 * i + 4
            ya = ya_psum.tile([128, 512], f32, tag="ya")

            def emit_attnv(j, lo, o, e):
                nc.tensor.matmul(
                    ya[:, o:512],
                    v_sb[:, j, 128 * h : 128 * h + 128],
                    e[:, lo : lo + 512 - o],
                    start=(j == 0),
                    stop=(j == nch - 1),
                    skip_group_check=True,
                )

            pend = deque()
            for jp in range(nch // 2):
                ps = sc_psum.tile([128, 1024], f32, tag="sc")
                e = e_pool.tile([128, 1024], bf16, tag="e")
                offs = []
                for hi in range(2):
                    j = 2 * jp + hi
                    s = j - 4 * i
                    o = 128 * s if s > 0 else 0
                    offs.append((j, o, s))
                    nc.tensor.matmul(
                        ps[:, 512 * hi + o : 512 * hi + 512],
                        k_sb[p0 : p0 + 64, blk, 128 * j : 128 * j + 128],
                        q_sb[p0 : p0 + 64, blk, 512 * i + o : 512 * i + 512],
                        start=True,
                        stop=True,
                    )
                if offs[0][2] < 0:
                    # full pair: one wide exp (each activation op carries a
                    # ~400ns fixed cost on the scalar engine)
                    nc.scalar.activation(e, ps, Exp, scale=0.125)
                else:
                    for hi, (j, o, s) in enumerate(offs):
                        lo = 512 * hi + o
                        nc.scalar.activation(
                            e[:, lo : lo + 512 - o], ps[:, lo : lo + 512 - o],
                            Exp, scale=0.125)
                        # causal mask: zero the exp of future keys
                        nc.vector.tensor_mul(
                            e[:, lo : lo + 512 - o], e[:, lo : lo + 512 - o],
                            pm_sb[:, 512 * s + o : 512 * s + 512],
                        )
                for hi, (j, o, s) in enumerate(offs):
                    pend.append((j, 512 * hi + o, o, e))
                    if len(pend) > 3:
                        emit_attnv(*pend.popleft())
            while pend:
                emit_attnv(*pend.popleft())
            return ya

        # ---- main loop ----
        for part in range(4):
            emit_proj_part(0, part)
        pending_norm = None
        pending_out = deque()
        for i in range(4):
            if i < 3:
                emit_xdma(i + 1, nc.gpsimd)
            for h in range(4):
                ya = attention(i, h)
                # wo staggered out of the startup DMA burst (HAM throttle)
                if i == 0 and h == 2:
                    for c in range(2):
                        nc.sync.dma_start(wo_sb[:, c, :], woM[:, c, :])
                # deferred work: never let the normalize chain or out-proj
                # head-of-line-block the PE stream of the current head
                if pending_norm is not None:
                    pending_norm()
                pending_norm = norm_closure(h, i, ya)
                if i < 3:
                    emit_proj_part(i + 1, h)
                # pop only out-proj blocks queued in EARLIER iterations: a
                # tile's tb0 must wait for its own head 3's normalize
                if pending_out:
                    pending_out.popleft()()
                if h == 3:
                    for tb in range(4):
                        pending_out.append(outproj_tb_closure(i, tb))
        pending_norm()
        while pending_out:
            pending_out.popleft()()

    nc.compile()
    return nc


def _get_program():
    if "nc" not in _CACHE:
        _CACHE["nc"] = _build_program()
    return _CACHE["nc"]


def _host_consts():
    import ml_dtypes

    bf16 = ml_dtypes.bfloat16
    # pairmask[s]: mask(o=128s)[p, t] = 1 iff query t >= key p + 128s
    p = np.arange(128)[:, None]
    t = np.arange(512)[None, :]
    pairmask = np.zeros((128, 2048), dtype=np.float32)
    for s in range(4):
        pairmask[:, 512 * s : 512 * s + 512] = (t >= p + 128 * s)
    bcsel = np.zeros((128, 256), dtype=np.float32)
    bcsel[64, 0:128] = 1.0   # even heads: broadcast denom row 64
    bcsel[0, 128:256] = 1.0  # odd heads: broadcast denom row 0
    return {
        "pairmask": pairmask.astype(bf16),
        "vcones": np.ones((128, 16), dtype=np.float32).astype(bf16),
        "bcsel": bcsel,
    }


def _enable_trace_support():
    """Best-effort: make trace=True work in this container (NTFF hook shim +
    disable artifact upload). No-op if anything is missing."""
    import sys
    import types

    try:
        import concourse.bass_utils as bu

        bu.upload_artifacts = lambda tmpdir: tmpdir
        try:
            from antenv.axon_hooks import get_axon_ntff_profile_hook  # noqa: F401

            return True
        except ImportError:
            pass
        import antenv
        from trn_agent_boot.trn_boot import _ntff_profile_via_ctypes

        hook = _ntff_profile_via_ctypes("/opt/axon/libaxon_pjrt.so")
        mod = types.ModuleType("antenv.axon_hooks")
        state = {"hook": hook}
        mod.get_axon_ntff_profile_hook = lambda: state["hook"]
        mod.set_axon_ntff_profile_hook = lambda h: state.__setitem__("hook", h)
        sys.modules["antenv.axon_hooks"] = mod
        antenv.axon_hooks = mod
        return hook is not None
    except Exception:
        return False


def kernel(x, attention_mask, Wq, Wk, Wv, Wo, bo):
    global LAST_RESULTS
    import ml_dtypes
    from concourse.bass_utils import run_bass_kernel_spmd

    x = np.asarray(x, dtype=np.float32)
    attention_mask = np.asarray(attention_mask, dtype=np.float32)
    Wq = np.asarray(Wq, dtype=np.float32)
    Wk = np.asarray(Wk, dtype=np.float32)
    Wv = np.asarray(Wv, dtype=np.float32)
    Wo = np.asarray(Wo, dtype=np.float32)
    bo = np.asarray(bo, dtype=np.float32)

    nc = _get_program()
    bf16 = ml_dtypes.bfloat16

    def pack(a, chunks):
        # [128*chunks, F] -> [128, chunks, F] grouped by 128-row chunk
        f = a.shape[1]
        return np.ascontiguousarray(
            a.reshape(chunks, 128, f).transpose(1, 0, 2)).astype(bf16)

    # host-side shard prep
    xm = x * attention_mask[:, :, None]
    xTs = [pack(xm[b].T, 8) for b in range(B)]
    consts = _host_consts()

    in_maps = []
    for core in range(8):
        b, g = divmod(core, 4)
        sl = slice(DLOC * g, DLOC * g + DLOC)
        in_maps.append(
            {
                "xT": xTs[b],
                "wqT": pack(np.ascontiguousarray(Wq[sl]).T, 8),
                "wkT": pack(np.ascontiguousarray(Wk[sl]).T, 8),
                "wvT": pack(np.ascontiguousarray(Wv[sl]).T, 8),
                "woM": pack(np.ascontiguousarray(Wo[:, sl]).T, 2),
                **consts,
            }
        )

    trace = bool(int(os.environ.get("KERNEL_TRACE", "0")))
    if trace:
        trace = _enable_trace_support()
    res = run_bass_kernel_spmd(nc, in_maps, core_ids=list(range(8)), trace=trace)
    LAST_RESULTS = res

    out = np.zeros((B, L, E), dtype=np.float32)
    for core in range(8):
        out[core // 4] += res.results[core]["outp"].astype(np.float32)
    out += bo
    return out
